# revision 15
# baseline (speedup 1.0000x reference)
"""AngularLayer Trainium2 kernel: [500000, 63] -> [500000, 483].

Per row: 21 (x,y) landmarks -> 210 ordered-pair unit direction vectors
(clipped x/y components), appended to the input row.

Sharded batch-parallel over 8 NeuronCores (62500 rows/core), SPMD one graph.
Layout per core: tiles of [125 partitions x 10 rows], features on the free
axis.  DVE does pair differences (interleaved (x,y) layout -- the only
fast DVE write/read order), ACT does squares + one dense rsqrt, GPSIMD
does clip+convert, DMA streams rows HBM<->SBUF.

Norm trick: instead of nsq[q] = sq[2q]+sq[2q+1] (strided, ~2.3 cyc/elem on
DVE) followed by TWO strided rsqrt duplications on ACT, compute the
DUPLICATED-interleaved norm in one dense 2x DVE add:
    nsqd[2q+c] = sq[2q+c] + sq[2q + (1-c)]
where the second operand is the same 32-bit word with its two bf16 halves
swapped (inner step -1, still a packed aligned read).  One dense rsqrt on
nsqd then yields rrd already duplicated per component, so the tilt multiply
is a single fully-dense bf16 2x tensor_tensor.

Measured on HW (8 cores, SPMD): 854 us (baseline interleaved-norm version:
886 us).  Rejected by measurement: planar vxy layouts (DVE scattered bf16
writes hit RMW, up to 4x slower subs), GPSIMD tail subtractions (GPSIMD
>35% busy taxes DVE via the shared SBUF port), R=20 tiles (in-place norm
loses the 2x DVE mode, 38KB DMA packets stream no faster than 19KB, and
the tighter buffering exposes the cross-engine chain), deeper software
pipelines (no gain; the 3-stage pipeline already overlaps ~95% of DVE).
"""

import os
from contextlib import ExitStack

import numpy as np

import concourse.bass as bass
import concourse.mybir as mybir
import concourse.tile as tile
from concourse import bacc
from concourse.bass_utils import run_bass_kernel_spmd

F32 = mybir.dt.float32
BF16 = mybir.dt.bfloat16
AF = mybir.ActivationFunctionType
ALU = mybir.AluOpType

N_CORES = 8
B_FULL = 500000
B_SHARD = B_FULL // N_CORES  # 62500
PARTS = 125
NLM = 21
NPAIR = 210
IN_C = 63
OUT_C = 483

ROWS_PER_PART = int(os.environ.get("ANGULAR_R", "10"))


def _build_nc(b_shard: int, rows_per_part: int) -> bass.Bass:
    R = rows_per_part
    assert b_shard % (PARTS * R) == 0
    n_tiles = b_shard // (PARTS * R)
    NF = R * 2 * NPAIR  # 4200 @ R=10: interleaved pair-component count

    nc = bacc.Bacc("TRN2", target_bir_lowering=False, debug=False)
    inp = nc.dram_tensor("tensor", [b_shard, IN_C], F32, kind="ExternalInput")
    outp = nc.dram_tensor("out", [b_shard, OUT_C], F32, kind="ExternalOutput")

    with tile.TileContext(nc) as tc, ExitStack() as ctx:
        opool = ctx.enter_context(tc.tile_pool(name="o", bufs=5))
        vpool = ctx.enter_context(tc.tile_pool(name="vxy", bufs=4))
        sqxp = ctx.enter_context(tc.tile_pool(name="sqx", bufs=2))
        npool = ctx.enter_context(tc.tile_pool(name="nsq", bufs=2))
        rrpool = ctx.enter_context(tc.tile_pool(name="rr", bufs=2))
        tpool = ctx.enter_context(tc.tile_pool(name="tt", bufs=2))

        st: dict = {}

        def stage_a(t):
            # DMA in + pair differences (interleaved (x,y), iteration order
            # [r, k, two]: innermost is the contiguous 8-byte (x,y) source
            # pair -- the only AP order the DVE runs at ~1 elem/cyc here)
            base = t * PARTS * R
            o = opool.tile([PARTS, R * OUT_C], F32, tag="o")
            o3 = o[:].rearrange("p (r c) -> p r c", c=OUT_C)

            # input loads into output tile's first 63 cols, both HWDGE
            # queues.  The runtime splits each DMA over E = largest
            # divisor(partition count) <= 16 SDMA engines -> use 60/64/1.
            src = inp[base:base + PARTS * R, :].rearrange(
                "(p r) c -> p r c", p=PARTS)
            nc.sync.dma_start(out=o3[0:60, :, 0:IN_C], in_=src[0:60].opt())
            nc.scalar.dma_start(out=o3[60:124, :, 0:IN_C], in_=src[60:124].opt())
            nc.sync.dma_start(out=o3[124:125, :, 0:IN_C], in_=src[124:125].opt())

            vxy = vpool.tile([PARTS, NF], BF16, tag="vxy")
            vxy4 = vxy[:].rearrange("p (r q two) -> p r q two", q=NPAIR, two=2)
            pb = 0
            for i in range(NLM - 1):
                np_i = NLM - 1 - i
                minu = o3[:, :, 3 * (i + 1):IN_C].rearrange(
                    "p r (k three) -> p r k three", three=3)[:, :, :, 0:2]
                subt = o3[:, :, 3 * i:3 * i + 2].unsqueeze(2).broadcast_to(
                    (PARTS, R, np_i, 2))
                nc.vector.tensor_sub(vxy4[:, :, pb:pb + np_i, :], minu, subt)
                pb += np_i
            st[t] = {"o": o, "o3": o3, "vxy": vxy}

        def stage_b1(t):
            # squares (ACT) -- first in the ACT queue each iteration, with
            # its input (subs(t)) already one full iteration old, so the
            # downstream swap-add never waits on it
            vxy = st[t]["vxy"]
            sq = sqxp.tile([PARTS, NF], BF16, tag="sqx")
            nc.scalar.activation(sq[:], vxy[:], AF.Square)
            st[t]["sq"] = sq

        def stage_b2(t):
            # duplicated norm (swap-add, dense 2x) -> one dense rsqrt
            sq = st[t].pop("sq")
            sqv = sq[:].rearrange("p (q two) -> p q two", two=2)
            nsqd = npool.tile([PARTS, NF], BF16, tag="nsq")
            nsqv = nsqd[:].rearrange("p (q two) -> p q two", two=2)
            nc.vector.tensor_add(nsqv, sqv, sqv[:, :, ::-1])

            rrd = rrpool.tile([PARTS, NF], BF16, tag="rr")
            nc.scalar.activation(rrd[:], nsqd[:], AF.Abs_reciprocal_sqrt)
            st[t]["rr"] = rrd

        def stage_b3(t):
            # dense 2x multiply, one iteration after its rsqrt
            vxy = st[t].pop("vxy")
            rrd = st[t].pop("rr")
            tt = tpool.tile([PARTS, NF], BF16, tag="tt")
            nc.vector.tensor_mul(tt[:], vxy[:], rrd[:])
            st[t]["tt"] = tt

        def stage_c(t):
            # clip + bf16->f32 [GPSIMD], DMA out
            base = t * PARTS * R
            o, o3, tt = (st[t][k] for k in ("o", "o3", "tt"))
            o_tilt = o3[:, :, IN_C:OUT_C]
            tt3 = tt[:].rearrange("p (r c) -> p r c", c=2 * NPAIR)
            nc.gpsimd.tensor_scalar(o_tilt, tt3, 1.0, -1.0, ALU.min, ALU.max)

            dst = outp[base:base + PARTS * R, :].rearrange(
                "(p r) c -> p (r c)", p=PARTS)
            nc.sync.dma_start(out=dst[0:60], in_=o[0:60, :])
            nc.scalar.dma_start(out=dst[60:124], in_=o[60:124, :])
            nc.sync.dma_start(out=dst[124:125], in_=o[124:125, :])
            del st[t]

        # 5-stage software pipeline: subs(t)@t, Square(t)@t+1,
        # swapadd+rsqrt(t)@t+2, mult(t)@t+3, clip+out(t)@t+4.  Every
        # cross-engine input is at least one full iteration old, so no
        # engine waits on another's same-iteration output (the 3-stage
        # version lost ~3us/tile to the add->rsqrt->Square lag loop).
        for s in range(n_tiles + 4):
            if s >= 4:
                stage_c(s - 4)
            if s < n_tiles:
                stage_a(s)
            if 1 <= s <= n_tiles:
                stage_b1(s - 1)
            if 2 <= s <= n_tiles + 1:
                stage_b2(s - 2)
            if 3 <= s <= n_tiles + 2:
                stage_b3(s - 3)

    nc.compile()
    return nc


_NC_CACHE: dict = {}


def _get_nc():
    key = (B_SHARD, ROWS_PER_PART)
    if key not in _NC_CACHE:
        _NC_CACHE[key] = _build_nc(B_SHARD, ROWS_PER_PART)
    return _NC_CACHE[key]


def kernel(tensor: np.ndarray) -> np.ndarray:
    tensor = np.ascontiguousarray(np.asarray(tensor, dtype=np.float32))
    assert tensor.shape == (B_FULL, IN_C), tensor.shape

    nc = _get_nc()
    in_maps = [
        {"tensor": tensor[c * B_SHARD:(c + 1) * B_SHARD]} for c in range(N_CORES)
    ]
    trace = os.environ.get("ANGULAR_TRACE", "0") == "1"
    res = run_bass_kernel_spmd(
        nc, in_maps, core_ids=list(range(N_CORES)), trace=trace
    )
    if trace:
        kernel.last_exec_time_ns = res.exec_time_ns
        kernel.last_results = res
    out = np.concatenate([res.results[c]["out"] for c in range(N_CORES)], axis=0)
    return out
